# revision 2
# baseline (speedup 1.0000x reference)
"""Trainium2 Bass kernel for the CSCG batched masked HMM forward pass.

Problem: for each of B=8 padded observation sequences, run a log-space HMM
forward recurrence restricted to 512-state clone blocks selected by
consecutive observation pairs, and read log P(obs) at true_len-1.

Strategy (one sequence per NeuronCore, 8 cores):
  * Work in linear space with a scalar log-offset instead of logsumexp:
        v_{t+1} = (v_t @ exp(blk_t)) * 2^e_k   (occasionally / c, tracked in L)
    The 512x512 linear matvec runs on the TensorEngine as 16 PSUM-accumulated
    (K=128, M=128, N=1) matmuls whose input/output layout is identical
    ([128 partitions = low 7 bits of state, 4 free = high 2 bits]), so the
    serial chain needs no transposes.
  * The host precomputes exp(log_T)*S in fp8e4 into a block-major input
    (256 blocks of 512x512 -> 128 rows x 2KB each), cutting steady-state
    streaming traffic 4x vs f32 and keeping the on-device program to just
    the recurrence.
  * Per step, one indirect DMA gathers the 256KB block for observation pair
    (o_{t-1}, o_t) using a host-precomputed row-index table.
  * Steps past true_len-1 multiply by a constant pad block that preserves
    sum(v) exactly, so all cores run a uniform step count and the final
    readout log(sum(v)) + L equals the value at true_len-1.
"""

import math
from contextlib import ExitStack

import numpy as np
import ml_dtypes

N_OBS = 16
C = 512
N_STATES = N_OBS * C  # 8192
B = 8
T = 1024
N_CORES = 8
PAD_BLOCK = N_OBS * N_OBS  # index of the constant pad block
N_TABLE_ROWS = (PAD_BLOCK + 1) * 128  # 33024 gather rows of 2048 bytes


def _build_bass(n_steps: int, e_k: int, renorm_every: int,
                renorm_defer: int, blk_bufs: int = 12, repeat_p2: int = 1):
    import concourse.bass as bass
    import concourse.tile as tile
    from concourse import bacc, mybir

    fp8 = mybir.dt.float8e4
    bf16 = mybir.dt.bfloat16
    f32 = mybir.dt.float32
    i32 = mybir.dt.int32
    Act = mybir.ActivationFunctionType

    k_copy = 2.0 ** e_k

    nc = bacc.Bacc(None, target_bir_lowering=False)
    offs_in = nc.dram_tensor("offs", [128, max(n_steps, 1)], i32, kind="ExternalInput")
    v0_in = nc.dram_tensor("v0", [128, 4], bf16, kind="ExternalInput")
    scratch = nc.dram_tensor("scratch", [N_TABLE_ROWS, 2048], fp8,
                             kind="ExternalInput")
    out_t = nc.dram_tensor("out", [1, 1], f32, kind="ExternalOutput")
    p_out = nc.dram_tensor("p_out", [128, 4], f32, kind="ExternalOutput")
    L_out = nc.dram_tensor("L_out", [1, 1], f32, kind="ExternalOutput")

    with ExitStack() as ctx:
        tc = ctx.enter_context(tile.TileContext(nc))

        pconst = ctx.enter_context(tc.tile_pool(name="pconst", bufs=1))
        pblk = ctx.enter_context(tc.tile_pool(name="pblk", bufs=blk_bufs))
        pp = ctx.enter_context(tc.tile_pool(name="pp", bufs=3))
        pscale = ctx.enter_context(tc.tile_pool(name="pscale", bufs=2))
        psmall = ctx.enter_context(tc.tile_pool(name="psmall", bufs=2))
        ps_v = ctx.enter_context(tc.tile_pool(name="ps_v", bufs=4, space="PSUM"))
        ps_c = ctx.enter_context(tc.tile_pool(name="ps_c", bufs=2, space="PSUM"))
        ps_b = ctx.enter_context(tc.tile_pool(name="ps_b", bufs=2, space="PSUM"))

        offs_sb = pconst.tile([128, max(n_steps, 1)], i32)
        nc.sync.dma_start(offs_sb[:], offs_in[:])

        ones_col = pconst.tile([128, 1], bf16)
        nc.vector.memset(ones_col[:], 1.0)
        ones_row = pconst.tile([1, 128], f32)
        nc.vector.memset(ones_row[:], 2.0 ** (-e_k))
        L_tile = pconst.tile([1, 1], f32)
        nc.vector.memset(L_tile[:], 0.0)

        for _p2 in range(repeat_p2):
            p_cur = pp.tile([128, 4], bf16, tag="p")
            nc.sync.dma_start(p_cur[:], v0_in[:])

            pending_scale = {}  # apply_step -> scale AP [128,1] with 2^e_k / c

            for k in range(1, n_steps + 1):
                blk = pblk.tile([128, 2048], fp8, tag="blk")
                nc.gpsimd.indirect_dma_start(
                    out=blk[:],
                    out_offset=None,
                    in_=scratch[:],
                    in_offset=bass.IndirectOffsetOnAxis(
                        ap=offs_sb[:, k - 1:k], axis=0),
                )

                psum = ps_v.tile([128, 4], f32, tag="v")
                for j_hi in range(4):
                    for i_hi in range(4):
                        nc.tensor.matmul(
                            out=psum[:, j_hi:j_hi + 1],
                            lhsT=blk[:, i_hi * 512 + j_hi * 128:
                                     i_hi * 512 + (j_hi + 1) * 128],
                            rhs=p_cur[:, i_hi:i_hi + 1],
                            start=(i_hi == 0),
                            stop=(i_hi == 3),
                        )

                p_next = pp.tile([128, 4], bf16, tag="p")
                if k in pending_scale:
                    nc.vector.tensor_scalar_mul(p_next[:], psum[:],
                                                pending_scale.pop(k))
                else:
                    nc.vector.tensor_scalar_mul(p_next[:], psum[:], k_copy)
                p_cur = p_next

                # Deferred global renorm: measure sum(p) now, apply a few
                # steps later so the reciprocal/broadcast chain stays off the
                # critical path; L accumulates log(c) to keep the readout
                # invariant.
                if renorm_every and k % renorm_every == 0 \
                        and k + renorm_defer <= n_steps:
                    c_ps = ps_c.tile([1, 4], f32, tag="c")
                    nc.tensor.matmul(out=c_ps[:], lhsT=ones_col[:],
                                     rhs=p_cur[:], start=True, stop=True)
                    c_sb = psmall.tile([1, 1], f32, tag="c_sb")
                    nc.vector.reduce_sum(c_sb[:], c_ps[:],
                                         axis=mybir.AxisListType.X)
                    bc_ps = ps_b.tile([128, 1], f32, tag="bc")
                    nc.tensor.matmul(out=bc_ps[:], lhsT=ones_row[:],
                                     rhs=c_sb[:], start=True, stop=True)
                    scale_sb = pscale.tile([128, 1], f32, tag="scale")
                    nc.vector.reciprocal(scale_sb[:], bc_ps[:])
                    lnc = psmall.tile([1, 1], f32, tag="lnc")
                    nc.scalar.activation(lnc[:], c_sb[:], Act.Ln)
                    nc.vector.tensor_add(L_tile[:], L_tile[:], lnc[:])
                    pending_scale[k + renorm_defer] = scale_sb[:, 0:1]

        # ---------------- Readout: log(sum(v)) + L ---------------------------
        f_ps = ps_c.tile([1, 4], f32, tag="c")
        nc.tensor.matmul(out=f_ps[:], lhsT=ones_col[:], rhs=p_cur[:],
                         start=True, stop=True)
        s_sb = psmall.tile([1, 1], f32, tag="c_sb")
        nc.vector.reduce_sum(s_sb[:], f_ps[:], axis=mybir.AxisListType.X)
        lns = psmall.tile([1, 1], f32, tag="lnc")
        nc.scalar.activation(lns[:], s_sb[:], Act.Ln)
        res = pscale.tile([1, 1], f32, tag="res")
        nc.vector.tensor_add(res[:], lns[:], L_tile[:])
        nc.sync.dma_start(out_t[:], res[:])
        p_f32 = pscale.tile([128, 4], f32, tag="p_f32")
        nc.vector.tensor_copy(p_f32[:], p_cur[:])
        nc.sync.dma_start(p_out[:], p_f32[:])
        nc.sync.dma_start(L_out[:], L_tile[:])

    nc.finalize()
    return nc


def _host_prep(log_T, log_pi, obs_batch, true_lens, n_steps):
    """fp8 block-major transition table, per-core offset tables, initial
    states, and readout constants."""
    log_T = np.asarray(log_T, dtype=np.float32)
    maxlog = float(np.max(log_T))
    ln_S = math.log(128.0) - maxlog  # max fp8 entry = 128

    # exp(log_T)*S in fp8, laid out block-major:
    #   scratch[(op*16+oc)*128 + i_lo, i_hi*512 + j]
    #     = exp(log_T)[op*512 + i_hi*128 + i_lo, oc*512 + j] * S
    scaled = np.exp(log_T + np.float32(ln_S))
    mean_rowsum = float(scaled.mean(dtype=np.float64)) * C
    e_k = int(np.clip(-round(math.log2(max(mean_rowsum, 1e-30))), -16, 0))
    kappa = 2.0 ** (-9 - e_k)  # pad-block entry; exact in fp8e4 for e_k in [-16, 0]

    scratch = np.empty((N_TABLE_ROWS, 2048), dtype=ml_dtypes.float8_e4m3)
    scratch[:PAD_BLOCK * 128] = (
        scaled.astype(ml_dtypes.float8_e4m3)
        .reshape(N_OBS, 4, 128, N_OBS, C)
        .transpose(0, 3, 2, 1, 4)
        .reshape(PAD_BLOCK * 128, 2048)
    )
    scratch[PAD_BLOCK * 128:] = np.float32(kappa)

    offs = np.empty((N_CORES, 128, max(n_steps, 1)), dtype=np.int32)
    v0 = np.empty((N_CORES, 128, 4), dtype=ml_dtypes.bfloat16)
    host_const = np.empty((N_CORES,), dtype=np.float64)
    part = np.arange(128, dtype=np.int32)[:, None]

    for b in range(N_CORES):
        o = np.asarray(obs_batch[b], dtype=np.int64)
        tl = int(true_lens[b])
        blocks = o[:-1] * N_OBS + o[1:]  # step k uses blocks[k-1]
        blocks = blocks[:n_steps].copy()
        blocks[max(tl - 1, 0):] = PAD_BLOCK
        if n_steps == 0:
            blocks = np.array([PAD_BLOCK], dtype=np.int64)
        offs[b] = blocks[None, :].astype(np.int32) * 128 + part

        a0 = np.asarray(log_pi[o[0] * C:(o[0] + 1) * C], dtype=np.float64)
        m0 = float(np.max(a0))
        v0[b] = np.exp(a0 - m0).reshape(4, 128).T.astype(ml_dtypes.bfloat16)
        n_real = min(max(tl - 1, 0), n_steps)  # pad steps contribute nothing
        host_const[b] = m0 - n_real * (ln_S + e_k * math.log(2.0))

    return scratch, e_k, offs, v0, host_const


def _run(log_T, log_pi, obs_batch, true_lens, n_steps=T - 1,
         renorm_every=6, renorm_defer=3, trace=False, blk_bufs=12,
         repeat_p2=1, n_calls=1):
    from concourse.bass_utils import run_bass_kernel_spmd

    log_pi = np.asarray(log_pi, dtype=np.float32)
    obs_batch = np.asarray(obs_batch)
    true_lens = np.asarray(true_lens)

    scratch, e_k, offs, v0, host_const = _host_prep(
        log_T, log_pi, obs_batch, true_lens, n_steps)

    nc = _build_bass(n_steps, e_k, renorm_every, renorm_defer, blk_bufs,
                     repeat_p2=repeat_p2)

    in_maps = [
        {"scratch": scratch, "offs": np.ascontiguousarray(offs[b]),
         "v0": np.ascontiguousarray(v0[b])}
        for b in range(N_CORES)
    ]
    import time as _time
    call_walls = []
    for _ in range(n_calls):
        t0 = _time.time()
        res = run_bass_kernel_spmd(nc, in_maps, core_ids=list(range(N_CORES)),
                                   trace=trace)
        call_walls.append(_time.time() - t0)
    res.call_walls = call_walls
    logZ = np.array(
        [res.results[b]["out"][0, 0] + host_const[b] for b in range(N_CORES)],
        dtype=np.float32,
    )
    return logZ, res


def kernel(log_T, log_pi, obs_batch, true_lens, n_clones=C, **_ignored):
    assert int(n_clones) == C, f"kernel hardcodes n_clones={C}, got {n_clones}"
    logZ, _ = _run(log_T, log_pi, obs_batch, true_lens)
    return logZ


# revision 23
# speedup vs baseline: 1.0009x; 1.0009x over previous
"""Trainium2 Bass kernel for the CSCG batched masked HMM forward pass.

Problem: for each of B=8 padded observation sequences, run a log-space HMM
forward recurrence restricted to 512-state clone blocks selected by
consecutive observation pairs, and read log P(obs) at true_len-1.

Strategy (one sequence per NeuronCore, 8 cores):
  * Work in linear space with a scalar log-offset instead of logsumexp:
        v_{t+1} = (v_t @ exp(blk_t)) * 2^e_k   (occasionally / c, tracked in L)
    The 512x512 linear matvec runs on the TensorEngine as 16 PSUM-accumulated
    (K=128, M=128, N=1) matmuls whose input/output layout is identical
    ([128 partitions = low 7 bits of state, 4 free = high 2 bits]), so the
    serial chain needs no transposes.
  * The host precomputes exp(log_T)*S in fp8e4 into a block-major input
    (256 blocks of 512x512 -> 128 rows x 2KB each), cutting steady-state
    streaming traffic 4x vs f32 and keeping the on-device program to just
    the recurrence.
  * Per step, one indirect DMA gathers the 256KB block for observation pair
    (o_{t-1}, o_t) using a host-precomputed row-index table.
  * Steps past true_len-1 multiply by a constant pad block that preserves
    sum(v) exactly, so all cores run a uniform step count and the final
    readout log(sum(v)) + L equals the value at true_len-1.
"""

import math
from contextlib import ExitStack

import numpy as np
import ml_dtypes

N_OBS = 16
C = 512
N_STATES = N_OBS * C  # 8192
B = 8
T = 1024
N_CORES = 8
PAD_BLOCK = N_OBS * N_OBS  # index of the constant pad block
N_TABLE_ROWS = (PAD_BLOCK + 1) * 128  # 33024 gather rows of 2048 bytes


def _build_bass(n_steps: int, e_k: int, renorm_every: int,
                renorm_defer: int, blk_bufs: int = 3, gather_g: int = 8,
                repeat_p2: int = 1):
    import concourse.bass as bass
    import concourse.tile as tile
    from concourse import bacc, mybir

    fp8 = mybir.dt.float8e4
    bf16 = mybir.dt.bfloat16
    f32 = mybir.dt.float32
    i32 = mybir.dt.int32
    Act = mybir.ActivationFunctionType

    k_copy = 2.0 ** e_k

    n_tab = ((max(n_steps, 1) + gather_g - 1) // gather_g) * gather_g

    nc = bacc.Bacc(None, target_bir_lowering=False)
    offs_in = nc.dram_tensor("offs", [128, n_tab], i32, kind="ExternalInput")
    v0_in = nc.dram_tensor("v0", [128, 4], bf16, kind="ExternalInput")
    scratch = nc.dram_tensor("scratch", [N_TABLE_ROWS, 2048], fp8,
                             kind="ExternalInput")
    out_t = nc.dram_tensor("out", [1, 1], f32, kind="ExternalOutput")
    p_out = nc.dram_tensor("p_out", [128, 4], f32, kind="ExternalOutput")
    L_out = nc.dram_tensor("L_out", [1, 1], f32, kind="ExternalOutput")

    with ExitStack() as ctx:
        tc = ctx.enter_context(tile.TileContext(nc))

        pconst = ctx.enter_context(tc.tile_pool(name="pconst", bufs=1))
        pblk = ctx.enter_context(tc.tile_pool(name="pblk", bufs=blk_bufs))
        pp = ctx.enter_context(tc.tile_pool(name="pp", bufs=3))
        pscale = ctx.enter_context(tc.tile_pool(name="pscale", bufs=2))
        psmall = ctx.enter_context(tc.tile_pool(name="psmall", bufs=2))
        ps_v = ctx.enter_context(tc.tile_pool(name="ps_v", bufs=4, space="PSUM"))
        ps_c = ctx.enter_context(tc.tile_pool(name="ps_c", bufs=2, space="PSUM"))
        ps_b = ctx.enter_context(tc.tile_pool(name="ps_b", bufs=2, space="PSUM"))

        offs_sb = pconst.tile([128, n_tab], i32)
        nc.sync.dma_start(offs_sb[:], offs_in[:])

        ones_col = pconst.tile([128, 1], bf16)
        nc.vector.memset(ones_col[:], 1.0)
        ones_row = pconst.tile([1, 128], f32)
        nc.vector.memset(ones_row[:], 2.0 ** (-e_k))
        L_tile = pconst.tile([1, 1], f32)
        nc.vector.memset(L_tile[:], 0.0)

        for _p2 in range(repeat_p2):
            p_cur = pp.tile([128, 4], bf16, tag="p")
            nc.sync.dma_start(p_cur[:], v0_in[:])

            pending_scale = {}  # apply_step -> scale AP [128,1] with 2^e_k / c

            G = gather_g
            blk_group = None
            for k in range(1, n_steps + 1):
                # One SWDGE indirect gather per G steps: the ~1us fixed Q7
                # cost amortizes across G blocks (128*G row descriptors per
                # instruction instead of 128). Gathers are ALWAYS full-width:
                # the offs table is padded to a multiple of G with pad-block
                # rows, because partially-sliced gathers corrupt on HW.
                g = (k - 1) % G
                if g == 0:
                    blk_group = pblk.tile([128, G * 2048], fp8, tag="blk")
                    nc.gpsimd.indirect_dma_start(
                        out=blk_group[:],
                        out_offset=None,
                        in_=scratch[:],
                        in_offset=bass.IndirectOffsetOnAxis(
                            ap=offs_sb[:, k - 1:k - 1 + G], axis=0),
                    )
                blk = blk_group[:, g * 2048:(g + 1) * 2048]

                psum = ps_v.tile([128, 4], f32, tag="v")
                for j_hi in range(4):
                    for i_hi in range(4):
                        nc.tensor.matmul(
                            out=psum[:, j_hi:j_hi + 1],
                            lhsT=blk[:, i_hi * 512 + j_hi * 128:
                                     i_hi * 512 + (j_hi + 1) * 128],
                            rhs=p_cur[:, i_hi:i_hi + 1],
                            start=(i_hi == 0),
                            stop=(i_hi == 3),
                        )

                p_next = pp.tile([128, 4], bf16, tag="p")
                if k in pending_scale:
                    nc.vector.tensor_scalar_mul(p_next[:], psum[:],
                                                pending_scale.pop(k))
                else:
                    nc.vector.tensor_scalar_mul(p_next[:], psum[:], k_copy)
                p_cur = p_next

                # Deferred global renorm: measure sum(p) now, apply a few
                # steps later so the reciprocal/broadcast chain stays off the
                # critical path; L accumulates log(c) to keep the readout
                # invariant.
                if renorm_every and k % renorm_every == 0 \
                        and k + renorm_defer <= n_steps:
                    c_ps = ps_c.tile([1, 4], f32, tag="c")
                    nc.tensor.matmul(out=c_ps[:], lhsT=ones_col[:],
                                     rhs=p_cur[:], start=True, stop=True)
                    c_sb = psmall.tile([1, 1], f32, tag="c_sb")
                    nc.vector.reduce_sum(c_sb[:], c_ps[:],
                                         axis=mybir.AxisListType.X)
                    bc_ps = ps_b.tile([128, 1], f32, tag="bc")
                    nc.tensor.matmul(out=bc_ps[:], lhsT=ones_row[:],
                                     rhs=c_sb[:], start=True, stop=True)
                    scale_sb = pscale.tile([128, 1], f32, tag="scale")
                    nc.vector.reciprocal(scale_sb[:], bc_ps[:])
                    lnc = psmall.tile([1, 1], f32, tag="lnc")
                    nc.scalar.activation(lnc[:], c_sb[:], Act.Ln)
                    nc.vector.tensor_add(L_tile[:], L_tile[:], lnc[:])
                    pending_scale[k + renorm_defer] = scale_sb[:, 0:1]

        # ---------------- Readout: log(sum(v)) + L ---------------------------
        f_ps = ps_c.tile([1, 4], f32, tag="c")
        nc.tensor.matmul(out=f_ps[:], lhsT=ones_col[:], rhs=p_cur[:],
                         start=True, stop=True)
        s_sb = psmall.tile([1, 1], f32, tag="c_sb")
        nc.vector.reduce_sum(s_sb[:], f_ps[:], axis=mybir.AxisListType.X)
        lns = psmall.tile([1, 1], f32, tag="lnc")
        nc.scalar.activation(lns[:], s_sb[:], Act.Ln)
        res = pscale.tile([1, 1], f32, tag="res")
        nc.vector.tensor_add(res[:], lns[:], L_tile[:])
        nc.sync.dma_start(out_t[:], res[:])
        p_f32 = pscale.tile([128, 4], f32, tag="p_f32")
        nc.vector.tensor_copy(p_f32[:], p_cur[:])
        nc.sync.dma_start(p_out[:], p_f32[:])
        nc.sync.dma_start(L_out[:], L_tile[:])

    nc.finalize()
    return nc


def _host_prep(log_T, log_pi, obs_batch, true_lens, n_steps, gather_g=8):
    """fp8 block-major transition table, per-core offset tables, initial
    states, and readout constants."""
    log_T = np.asarray(log_T, dtype=np.float32)
    maxlog = float(np.max(log_T))
    ln_S = math.log(128.0) - maxlog  # max fp8 entry = 128

    # exp(log_T)*S in fp8, laid out block-major:
    #   scratch[(op*16+oc)*128 + i_lo, i_hi*512 + j]
    #     = exp(log_T)[op*512 + i_hi*128 + i_lo, oc*512 + j] * S
    scaled = np.exp(log_T + np.float32(ln_S))
    mean_rowsum = float(scaled.mean(dtype=np.float64)) * C
    e_k = int(np.clip(-round(math.log2(max(mean_rowsum, 1e-30))), -16, 0))
    kappa = 2.0 ** (-9 - e_k)  # pad-block entry; exact in fp8e4 for e_k in [-16, 0]

    scratch = np.empty((N_TABLE_ROWS, 2048), dtype=ml_dtypes.float8_e4m3)
    scratch[:PAD_BLOCK * 128] = (
        scaled.astype(ml_dtypes.float8_e4m3)
        .reshape(N_OBS, 4, 128, N_OBS, C)
        .transpose(0, 3, 2, 1, 4)
        .reshape(PAD_BLOCK * 128, 2048)
    )
    scratch[PAD_BLOCK * 128:] = np.float32(kappa)

    # offset table padded to a multiple of gather_g so every indirect
    # gather is full-width (partial gathers corrupt on HW)
    n_tab = ((max(n_steps, 1) + gather_g - 1) // gather_g) * gather_g
    offs = np.empty((N_CORES, 128, n_tab), dtype=np.int32)
    v0 = np.empty((N_CORES, 128, 4), dtype=ml_dtypes.bfloat16)
    host_const = np.empty((N_CORES,), dtype=np.float64)
    part = np.arange(128, dtype=np.int32)[:, None]

    for b in range(N_CORES):
        o = np.asarray(obs_batch[b], dtype=np.int64)
        tl = int(true_lens[b])
        blocks = o[:-1] * N_OBS + o[1:]  # step k uses blocks[k-1]
        blocks = blocks[:n_steps].copy()
        blocks[max(tl - 1, 0):] = PAD_BLOCK
        if n_steps == 0:
            blocks = np.array([PAD_BLOCK], dtype=np.int64)
        blocks = np.concatenate(
            [blocks, np.full(n_tab - len(blocks), PAD_BLOCK, dtype=np.int64)])
        offs[b] = blocks[None, :].astype(np.int32) * 128 + part

        a0 = np.asarray(log_pi[o[0] * C:(o[0] + 1) * C], dtype=np.float64)
        m0 = float(np.max(a0))
        v0[b] = np.exp(a0 - m0).reshape(4, 128).T.astype(ml_dtypes.bfloat16)
        n_real = min(max(tl - 1, 0), n_steps)  # pad steps contribute nothing
        host_const[b] = m0 - n_real * (ln_S + e_k * math.log(2.0))

    return scratch, e_k, offs, v0, host_const


def _run(log_T, log_pi, obs_batch, true_lens, n_steps=T - 1,
         renorm_every=6, renorm_defer=3, trace=False, blk_bufs=8,
         gather_g=1, repeat_p2=1, n_calls=1):
    from concourse.bass_utils import run_bass_kernel_spmd

    log_pi = np.asarray(log_pi, dtype=np.float32)
    obs_batch = np.asarray(obs_batch)
    true_lens = np.asarray(true_lens)

    scratch, e_k, offs, v0, host_const = _host_prep(
        log_T, log_pi, obs_batch, true_lens, n_steps, gather_g=gather_g)

    nc = _build_bass(n_steps, e_k, renorm_every, renorm_defer, blk_bufs,
                     gather_g=gather_g, repeat_p2=repeat_p2)

    in_maps = [
        {"scratch": scratch, "offs": np.ascontiguousarray(offs[b]),
         "v0": np.ascontiguousarray(v0[b])}
        for b in range(N_CORES)
    ]
    import time as _time
    call_walls = []
    for _ in range(n_calls):
        t0 = _time.time()
        res = run_bass_kernel_spmd(nc, in_maps, core_ids=list(range(N_CORES)),
                                   trace=trace)
        call_walls.append(_time.time() - t0)
    res.call_walls = call_walls
    logZ = np.array(
        [res.results[b]["out"][0, 0] + host_const[b] for b in range(N_CORES)],
        dtype=np.float32,
    )
    return logZ, res


def kernel(log_T, log_pi, obs_batch, true_lens, n_clones=C, **_ignored):
    assert int(n_clones) == C, f"kernel hardcodes n_clones={C}, got {n_clones}"
    logZ, _ = _run(log_T, log_pi, obs_batch, true_lens)
    return logZ
